# revision 7
# baseline (speedup 1.0000x reference)
"""Trainium2 Bass kernel for nn_ContactPredictionHead.

Reference computation (B=2, L=2048, D=1536, T=2):
    Wp, Wd = W[:, :D], W[:, D:]
    prod[b,i,j,t] = sum_d h[b,i,d] * Wp[t,d] * h[b,j,d]
    diff[b,i,j,t] = (h@Wd.T)[b,i,t] - (h@Wd.T)[b,j,t]
    out = symmetrize(prod + diff + bias)

Key identity: prod is symmetric in (i,j) and diff is antisymmetric, so the
symmetrization leaves   out[b,i,j,t] = prod[b,i,j,t] + bias[t]   exactly —
a weighted Gram matrix.  Only the upper triangle is computed on device; the
host mirrors the strict lower triangle.

Sharding: 4 cores per batch item.  The 16 row-blocks (128 rows each) of a
batch's L x L Gram matrix are dealt by a Latin square: core cc's stationary
slot s holds row-block I = 4s + ((s+cc)%4).  Slot s covers its arc
[128I, 2048) as one cc-dependent "partial" group [128I, 512(s+1)) plus
(3-s) full 512-col "static" groups — so every core computes exactly
4352 moving columns per t (the balanced ideal).  Static groups are
identical on all cores; the partial groups' offsets/sizes live in a 4-way
partition-id branch on the Tensor engine only (all other engines run
straight-line code: PSUM accs are padded to 512 and the host slices each
group's valid columns).

Phases run in REVERSE chunk order (3,2,1,0): per-phase matmul work then
always exceeds the per-chunk stream time, so the PE can never starve on
the input stream after phase 3's opening.

Slot 0's weighted stationaries are PRE-MULTIPLIED ON THE HOST ("hs0",
contiguous p-major, full DMA efficiency): the first matmuls gate only on
a small contiguous DMA, not on window DMA + on-device vector prep.
Slots 1-3 arrive as raw windows ("hw", p-major contiguous) and are
prepped on device: t0 products on the Vector engine, t1 products on the
Scalar engine (activation-with-scale), so both t's are ready together.

The PE clock is HAM-throttled to 1.2 GHz until ~3.4us of SUSTAINED
activity; the warmup block issues 512-col dummy matmuls on a
locally-memset tile to bridge from kernel start to first real matmul so
real matmuls run at 2.4 GHz from the first one.

All tensors stream as bfloat16 (PSUM accumulates fp32), halving DMA
volume vs fp32 at the same PE rate.  The final (v=0) partial's stores are
split into 128-col chunks so the post-last-matmul store drain is short.
"""
import sys

sys.path.insert(0, "/opt/trn_rl_repo")

import numpy as np
import ml_dtypes

BF16 = ml_dtypes.bfloat16

B, L, D, T = 2, 2048, 1536, 2
NCORES = 8
CPB = NCORES // B     # cores per batch item = 4
NK = D // 128         # contraction k-tiles = 12
NJ = 512              # j columns per full matmul (one PSUM bank of fp32)
NNB = L // NJ         # j chunks = 4
NS = 4                # stationary row slots per core (128 rows each)

PHASES = [3, 2, 1, 0]           # chunk/phase order (heaviest work first)
PREPS = [1, 2, 3]               # on-device prepped slots (slot 0 is host-prepped)
KH = 6                          # k-half size (hs0 parts and chunk halves)
KPARTS = {3: [(0, 6), (6, 12)],
          2: [(0, 6), (6, 12)],
          1: [(0, 6), (6, 12)],
          0: [(0, 6), (6, 12)]}
WARMUP_MMS = 9                  # 512-col dummies; cold ~427ns each => ~3.8us


def row_of(s, cc):
    """Global 128-row block held by slot s on a core with variant cc."""
    return 4 * s + (s + cc) % 4


def groups_of(cc):
    """Schedule (shared shape, variant-dependent geometry): list of
    (I, colstart, F) in emission order; 20 groups = [statics (s<v) t0, t1,
    then partial t0, t1] per chunk phase v, phases in PHASES order."""
    gs = []
    for v in PHASES:
        q = (v + cc) % 4
        for s in range(v):
            for _t in range(T):
                gs.append((row_of(s, cc), NJ * v, NJ))
        for _t in range(T):
            gs.append((row_of(v, cc), NJ * v + 128 * q, NJ - 128 * q))
    return gs


NG = len(groups_of(0))    # 20

_CACHE = {}


def _get_nc():
    if "nc" in _CACHE:
        return _CACHE["nc"]
    import concourse.tile as tile
    from concourse.tile_rust import add_dep_helper
    from concourse import bacc, mybir

    f32, bf16 = mybir.dt.float32, mybir.dt.bfloat16
    act_copy = mybir.ActivationFunctionType.Copy
    nc = bacc.Bacc("TRN2", target_bir_lowering=False, debug=False,
                   num_devices=NCORES, enable_partition_id=True,
                   enable_asserts=False)
    ht_d = nc.dram_tensor("ht", [D, L], bf16, kind="ExternalInput")
    # hw[p, (s-1, k, r)] = ht[128k+p, 128*row_of(s,cc) + r]  for s in 1..3
    hw_d = nc.dram_tensor("hw", [128, 3 * NK * 128], bf16,
                          kind="ExternalInput")
    # hs0[p, (kh, t, kk, r)] = ht[128(6kh+kk)+p, 128cc + r] * Wp[t, ...]
    hs0_d = nc.dram_tensor("hs0", [128, 2 * T * KH * 128], bf16,
                           kind="ExternalInput")
    wp_d = nc.dram_tensor("wp", [128, T * NK], bf16, kind="ExternalInput")
    # fp32 copy of wp for the scalar-engine activation scale (verifier
    # requires FP32 scale APs); 12KB, negligible.
    wpf_d = nc.dram_tensor("wpf", [128, T * NK], f32, kind="ExternalInput")
    out_d = nc.dram_tensor("out", [NG, 128, NJ], bf16, kind="ExternalOutput")

    with tile.TileContext(nc) as tc:
        with tc.tile_pool(name="big", bufs=1) as big, \
             tc.tile_pool(name="st", bufs=4) as stp, \
             tc.tile_pool(name="ps", bufs=4, space="PSUM") as psp, \
             tc.tile_pool(name="psw", bufs=1, space="PSUM") as psw:
            # wt[p, t*NK+k] = Wp[t, 128k+p] (pre-gathered on the host)
            wt = big.tile([128, T * NK], bf16, name="wt")
            wtf = big.tile([128, T * NK], f32, name="wtf")
            # hst0[p, kh, t, kk, r]: slot-0 stationaries (host premultiplied)
            hst0 = big.tile([128, 2, T, KH, 128], bf16, name="hst0")
            # hst[p, s-1, t, k, r] = htw[p, s-1, k, r] * Wp[t, 128k+p]
            hst = big.tile([128, 3, T, NK, 128], bf16, name="hst")
            # htw[p, s-1, k, r] = raw slot windows for s in 1..3
            htw = big.tile([128, 3, NK, 128], bf16, name="htw")
            # htall[p, k, j] = ht[128k+p, j]  (canonical, un-rolled)
            htall = big.tile([128, NK, L], bf16, name="htall")

            def s0ap(t, k):
                return hst0[:, k // KH, t, k % KH]

            def sap(s, t, k):
                return hst[:, s - 1, t, k]

            # Partition-id register load costs ~1.4us of queue time — issue
            # it first so it overlaps the framework preamble and DMA issues
            # instead of delaying the branch evaluations later.
            pid = nc.tensor.partition_id()
            cc = pid % 4

            nc.scalar.dma_start(wt[:], wp_d.ap())
            nc.scalar.dma_start(wtf[:], wpf_d.ap())
            # Warm the PE clock (HAM un-throttles after ~3.4 us of SUSTAINED
            # activity) with 512-col throwaway matmuls on a
            # locally-initialized scratch tile — no DMA dependency, so
            # warmup bridges from the preamble to the first real matmul.
            wdum = big.tile([128, NJ], bf16, name="wdum")
            nc.gpsimd.memset(wdum[:], 0.0)
            wacc = psw.tile([128, NJ], f32, name="wacc")
            for _ in range(WARMUP_MMS):
                nc.tensor.matmul(wacc[:], wdum[:, 0:128],
                                 wdum[:], start=True, stop=True)

            # Input stream, strictly ordered on the sync ring in consumption
            # order.
            prev = None

            def chain(dma):
                nonlocal prev
                if prev is not None:
                    add_dep_helper(dma.ins, prev.ins, sync=False,
                                   reason="input stream in consumption order")
                prev = dma

            def hs0_dma(kh):
                chain(nc.sync.dma_start(
                    hst0[:, kh],
                    hs0_d.ap()[:, kh * T * KH * 128:(kh + 1) * T * KH * 128]))

            def win_dma(s):
                chain(nc.sync.dma_start(
                    htw[:, s - 1],
                    hw_d.ap()[:, (s - 1) * NK * 128:s * NK * 128]))

            def chunk_part(v, k0, k1):
                chain(nc.sync.dma_start(
                    htall[:, k0:k1, v * NJ:(v + 1) * NJ],
                    ht_d.ap()[k0 * 128:k1 * 128, v * NJ:(v + 1) * NJ]
                    .rearrange("(k p) j -> p k j", p=128)))

            # Stream order: slot-0 stationaries and chunk-3 k-halves lead
            # (they gate the first matmuls), win1 rides between the halves
            # (its prep must finish by the time the s=1 statics start),
            # then win2/win3, then chunks 2, 1, 0.
            hs0_dma(0)
            chunk_part(3, 0, KH)
            win_dma(1)
            hs0_dma(1)
            chunk_part(3, KH, NK)
            win_dma(2)
            win_dma(3)
            for v in PHASES[1:]:
                for k0, k1 in KPARTS[v]:
                    chunk_part(v, k0, k1)

            # Stationary prep for slots 1..3 (straight-line; per-core rows
            # arrive via hw): t0 on Vector (tensor_mul with broadcast
            # scale), t1 on Scalar (per-k activation copy with scale), so
            # both t's of a slot are ready in parallel.
            for s in PREPS:
                scale = (wt[:, 0:NK].unsqueeze(2)
                         .broadcast_to([128, NK, 128]))
                nc.vector.tensor_mul(hst[:, s - 1, 0], htw[:, s - 1], scale)
                for k in range(NK):
                    nc.scalar.activation(
                        hst[:, s - 1, 1, k], htw[:, s - 1, k], act_copy,
                        scale=wtf[:, NK + k:NK + k + 1])

            def emit_partial(v, accs, ccv):
                q = (v + ccv) % 4
                off, fw = NJ * v + 128 * q, NJ - 128 * q
                stat = (lambda t, k: s0ap(t, k)) if v == 0 else \
                       (lambda t, k: sap(v, t, k))
                if v == 0:
                    # Final phase: split into 128-col chunks (separate PSUM
                    # accumulation groups) so stores drain progressively and
                    # the post-last-matmul chain is short.
                    nch = fw // 128
                    for t in range(T):
                        for c in range(nch):
                            for k in range(NK):
                                nc.tensor.matmul(
                                    accs[t][:, 128 * c:128 * (c + 1)],
                                    stat(t, k),
                                    htall[:, k, off + 128 * c:
                                          off + 128 * (c + 1)],
                                    start=(k == 0), stop=(k == NK - 1))
                else:
                    for t in range(T):
                        for k in range(NK):
                            nc.tensor.matmul(
                                accs[t][:, 0:fw], stat(t, k),
                                htall[:, k, off:off + fw],
                                start=(k == 0), stop=(k == NK - 1))

            def emit_static(s, v, acc, t):
                for k in range(NK):
                    nc.tensor.matmul(
                        acc[:], sap(s, t, k),
                        htall[:, k, v * NJ:(v + 1) * NJ],
                        start=(k == 0), stop=(k == NK - 1))

            def store(gi, acc):
                # Copies on scalar: the vector queue must stay free for the
                # stationary preps (a copy would trap a later prep behind a
                # matmul completion).  The gpsimd software queue has a
                # ~2.7us teardown drain, so it only carries stores that
                # complete well before the end.
                st = stp.tile([128, NJ], bf16, name="st", tag="st")
                nc.scalar.copy(st[:], acc[:])
                nc.gpsimd.dma_start(out_d.ap()[gi], st[:])

            def store_final(gi, acc, engsel):
                # 128-col chunk stores for the v=0 partial groups; the last
                # chunks ride vector copy + sync DMA (sync's teardown drain
                # is ~8ns) so the final chain after the last matmul is one
                # short copy + one small DMA.
                for c in range(4):
                    colr = slice(128 * c, 128 * (c + 1))
                    sth = stp.tile([128, 128], bf16, name="stf", tag="stf")
                    if engsel(c) == 0:
                        nc.scalar.copy(sth[:], acc[:, colr])
                        nc.scalar.dma_start(out_d.ap()[gi][:, colr], sth[:])
                    else:
                        nc.vector.tensor_copy(sth[:], acc[:, colr])
                        nc.sync.dma_start(out_d.ap()[gi][:, colr], sth[:])

            gi = 0
            for v in PHASES:
                first_s = 0
                if v > 0:
                    # Each phase's opening static pair can be paced by its
                    # chunk's stream: interleaving t0/t1 by k-half (two PSUM
                    # banks accumulating concurrently) matches consumption
                    # to arrival — free when the data is already resident.
                    accs0 = [psp.tile([128, NJ], f32, name="acc", tag="acc")
                             for _t in range(T)]
                    for k0, k1 in KPARTS[v]:
                        for t in range(T):
                            for k in range(k0, k1):
                                nc.tensor.matmul(
                                    accs0[t][:], s0ap(t, k),
                                    htall[:, k, v * NJ:(v + 1) * NJ],
                                    start=(k == 0), stop=(k == NK - 1))
                    for t in range(T):
                        store(gi, accs0[t])
                        gi += 1
                    first_s = 1
                for s in range(first_s, v):
                    for t in range(T):
                        acc = psp.tile([128, NJ], f32, name="acc", tag="acc")
                        emit_static(s, v, acc, t)
                        store(gi, acc)
                        gi += 1
                accs = [psp.tile([128, NJ], f32, name="acc", tag="acc")
                        for _t in range(T)]
                with tc.If(cc <= 1) as c1:
                    with tc.If(cc == 0) as c2:
                        emit_partial(v, accs, 0)
                    with c2.Else():
                        emit_partial(v, accs, 1)
                with c1.Else():
                    with tc.If(cc == 2) as c3:
                        emit_partial(v, accs, 2)
                    with c3.Else():
                        emit_partial(v, accs, 3)
                if v == 0:
                    # t0 chunks via scalar, t1 chunks alternate so the very
                    # last chunk drains on vector+sync.
                    store_final(gi, accs[0], lambda c: 0)
                    gi += 1
                    store_final(gi, accs[1], lambda c: 0 if c < 2 else 1)
                    gi += 1
                else:
                    for t in range(T):
                        store(gi, accs[t])
                        gi += 1
    nc.compile()
    _CACHE["nc"] = nc
    return nc


def make_in_maps(h, W):
    # wp[p, t*NK+k] = Wp[t, 128k+p]
    Wp = W[:, :D].astype(np.float32)
    wp = np.ascontiguousarray(
        Wp.reshape(T, NK, 128).transpose(2, 0, 1)
        .reshape(128, T * NK)).astype(BF16)
    hts = [np.ascontiguousarray(h[bi].T).astype(BF16) for bi in range(B)]
    # wpk[t, k, p] for the host-side slot-0 premultiply
    wpk = Wp.reshape(T, NK, 128)
    in_maps = []
    for c in range(NCORES):
        bi, cc = c // CPB, c % CPB
        ht32 = hts[bi].astype(np.float32)
        # hw[p, (s-1, k, r)] p-major contiguous windows for slots 1..3
        hw = np.empty((128, 3, NK, 128), np.float32)
        for s in (1, 2, 3):
            I = row_of(s, cc)
            # ht32[128k+p, 128I+r] -> [k, p, r] -> [p, k, r]
            win = ht32[:, 128 * I:128 * I + 128].reshape(NK, 128, 128)
            hw[:, s - 1] = win.transpose(1, 0, 2)
        hw = hw.reshape(128, 3 * NK * 128).astype(BF16)
        # hs0[p, (kh, t, kk, r)] = window0[k, p, r] * wpk[t, k, p]
        I0 = row_of(0, cc)
        win0 = ht32[:, 128 * I0:128 * I0 + 128].reshape(NK, 128, 128)
        hs0 = win0[None, :, :, :] * wpk[:, :, :, None]        # [t, k, p, r]
        hs0 = hs0.reshape(T, 2, KH, 128, 128)                  # [t, kh, kk, p, r]
        hs0 = hs0.transpose(3, 1, 0, 2, 4)                     # [p, kh, t, kk, r]
        hs0 = np.ascontiguousarray(hs0).reshape(
            128, 2 * T * KH * 128).astype(BF16)
        in_maps.append({"ht": hts[bi],
                        "hw": np.ascontiguousarray(hw),
                        "hs0": hs0, "wp": wp,
                        "wpf": np.ascontiguousarray(
                            wp.astype(np.float32))})
    return in_maps


def kernel(hidden_states, W, b):
    from concourse.bass_utils import run_bass_kernel_spmd

    h = np.ascontiguousarray(hidden_states, dtype=np.float32)
    W = np.asarray(W, dtype=np.float32)
    bias = np.asarray(b, dtype=np.float32)
    nc = _get_nc()

    res = run_bass_kernel_spmd(nc, make_in_maps(h, W),
                               core_ids=list(range(NCORES)))
    full = np.empty((B, L, L, T), np.float32)
    for c in range(NCORES):
        bi, cc = c // CPB, c % CPB
        blocks = np.asarray(res.results[c]["out"]).astype(np.float32)
        for gi, (I, colstart, fw) in enumerate(groups_of(cc)):
            t = gi % T
            rows = slice(128 * I, 128 * I + 128)
            full[bi, rows, colstart:colstart + fw, t] = blocks[gi, :, 0:fw]
    # Mirror: keep computed j >= i, take j < i from the transpose.
    idx = np.arange(L)
    mask = (idx[None, :] >= idx[:, None])[None, :, :, None]
    out = np.where(mask, full, full.transpose(0, 2, 1, 3))
    if np.any(bias != 0):
        out += bias
    return out


# revision 8
# speedup vs baseline: 1.2512x; 1.2512x over previous
"""Trainium2 Bass kernel for nn_ContactPredictionHead.

Reference computation (B=2, L=2048, D=1536, T=2):
    Wp, Wd = W[:, :D], W[:, D:]
    prod[b,i,j,t] = sum_d h[b,i,d] * Wp[t,d] * h[b,j,d]
    diff[b,i,j,t] = (h@Wd.T)[b,i,t] - (h@Wd.T)[b,j,t]
    out = symmetrize(prod + diff + bias)

Key identity: prod is symmetric in (i,j) and diff is antisymmetric, so the
symmetrization leaves   out[b,i,j,t] = prod[b,i,j,t] + bias[t]   exactly —
a weighted Gram matrix.  Only the upper triangle is computed on device; the
host mirrors the strict lower triangle.

Sharding: 4 cores per batch item.  The 16 row-blocks (128 rows each) of a
batch's L x L Gram matrix are dealt by a Latin square: core cc's stationary
slot s holds row-block I = 4s + ((s+cc)%4).  Slot s covers its arc
[128I, 2048) as one cc-dependent "partial" group [128I, 512(s+1)) plus
(3-s) full 512-col "static" groups — so every core computes exactly
4352 moving columns per t (the balanced ideal).  Static groups are
identical on all cores; the partial groups' offsets/sizes live in a 4-way
partition-id branch on the Tensor engine only (all other engines run
straight-line code: PSUM accs are padded to 512 and the host slices each
group's valid columns).

Phases run in REVERSE chunk order (3,2,1,0): per-phase matmul work then
always exceeds the per-chunk stream time, so the PE can never starve on
the input stream after phase 3's opening.

ALL weighted stationaries (slot window x Wp[t]) are PRE-MULTIPLIED ON THE
HOST and shipped p-major contiguous, so no on-device prep exists at all:
every matmul gates only on plain DMA arrivals, and the Vector/Scalar
engines carry nothing but output copies.  Slot 0 (which feeds every
phase's opening statics) is k-quartered so the first matmul starts on
~0.5MB of stream.

The PE clock is HAM-throttled to 1.2 GHz until ~3.4us of SUSTAINED
activity; the warmup block issues 512-col dummy matmuls (one chained
accumulation group, so they issue back-to-back) on a locally-memset tile
to bridge from kernel start to the first real matmul.

All tensors stream as bfloat16 (PSUM accumulates fp32), halving DMA
volume vs fp32 at the same PE rate.  The final (v=0) partial's stores are
split into 128-col chunks so the post-last-matmul store drain is short.
"""
import sys

sys.path.insert(0, "/opt/trn_rl_repo")

import numpy as np
import ml_dtypes

BF16 = ml_dtypes.bfloat16

B, L, D, T = 2, 2048, 1536, 2
NCORES = 8
CPB = NCORES // B     # cores per batch item = 4
NK = D // 128         # contraction k-tiles = 12
NJ = 512              # j columns per full matmul (one PSUM bank of fp32)
NNB = L // NJ         # j chunks = 4
NS = 4                # stationary row slots per core (128 rows each)

PHASES = [3, 2, 1, 0]           # chunk/phase order (heaviest work first)
KPARTS = {3: [(0, 3), (3, 6), (6, 9), (9, 12)],
          2: [(0, 6), (6, 12)],
          1: [(0, 6), (6, 12)],
          0: [(0, 6), (6, 12)]}
WARMUP_MMS = 6                  # 512-col dummies, one accumulation group


def row_of(s, cc):
    """Global 128-row block held by slot s on a core with variant cc."""
    return 4 * s + (s + cc) % 4


def groups_of(cc):
    """Schedule (shared shape, variant-dependent geometry): list of
    (I, colstart, F) in emission order; 20 groups = [statics (s<v) t0, t1,
    then partial t0, t1] per chunk phase v, phases in PHASES order."""
    gs = []
    for v in PHASES:
        q = (v + cc) % 4
        for s in range(v):
            for _t in range(T):
                gs.append((row_of(s, cc), NJ * v, NJ))
        for _t in range(T):
            gs.append((row_of(v, cc), NJ * v + 128 * q, NJ - 128 * q))
    return gs


NG = len(groups_of(0))    # 20

_CACHE = {}


def _get_nc():
    if "nc" in _CACHE:
        return _CACHE["nc"]
    import concourse.tile as tile
    from concourse.tile_rust import add_dep_helper
    from concourse import bacc, mybir

    f32, bf16 = mybir.dt.float32, mybir.dt.bfloat16
    nc = bacc.Bacc("TRN2", target_bir_lowering=False, debug=False,
                   num_devices=NCORES, enable_partition_id=True,
                   enable_asserts=False)
    ht_d = nc.dram_tensor("ht", [D, L], bf16, kind="ExternalInput")
    # hs0[p, (q, t, kk, r)]: slot-0 stationaries, k-quartered (q=k//3):
    #   ht[128(3q+kk)+p, 128*row_of(0,cc)+r] * Wp[t, 128(3q+kk)+p]
    hs0_d = nc.dram_tensor("hs0", [128, 4 * T * 3 * 128], bf16,
                           kind="ExternalInput")
    # hs[p, (s-1, t, k, r)]: slot 1..3 stationaries (host premultiplied)
    hs_d = nc.dram_tensor("hs", [128, 3 * T * NK * 128], bf16,
                          kind="ExternalInput")
    out_d = nc.dram_tensor("out", [NG, 128, NJ], bf16, kind="ExternalOutput")

    with tile.TileContext(nc) as tc:
        with tc.tile_pool(name="big", bufs=1) as big, \
             tc.tile_pool(name="st", bufs=4) as stp, \
             tc.tile_pool(name="ps", bufs=4, space="PSUM") as psp, \
             tc.tile_pool(name="psw", bufs=1, space="PSUM") as psw:
            # hst0[p, q, t, kk, r]: slot-0 stationaries (k-quartered)
            hst0 = big.tile([128, 4, T, 3, 128], bf16, name="hst0")
            # hss[p, s-1, t, k, r]: slot 1..3 stationaries
            hss = big.tile([128, 3, T, NK, 128], bf16, name="hss")
            # htall[p, k, j] = ht[128k+p, j]  (canonical, un-rolled)
            htall = big.tile([128, NK, L], bf16, name="htall")

            def s0ap(t, k):
                return hst0[:, k // 3, t, k % 3]

            def sap(s, t, k):
                return hss[:, s - 1, t, k]

            # Partition-id register load costs ~1.4us of queue time — issue
            # it first so it overlaps the framework preamble and DMA issues
            # instead of delaying the branch evaluations later.
            pid = nc.tensor.partition_id()
            cc = pid % 4

            # Warm the PE clock (HAM un-throttles after ~3.4 us of SUSTAINED
            # activity) with 512-col throwaway matmuls on a
            # locally-initialized scratch tile — no DMA dependency, so
            # warmup bridges from the preamble to the first real matmul.
            # One accumulation group: back-to-back issue, no PSUM WAW stalls.
            wdum = big.tile([128, NJ], bf16, name="wdum")
            nc.gpsimd.memset(wdum[:], 0.0)
            wacc = psw.tile([128, NJ], f32, name="wacc")
            for i in range(WARMUP_MMS):
                nc.tensor.matmul(wacc[:], wdum[:, 0:128], wdum[:],
                                 start=(i == 0), stop=(i == WARMUP_MMS - 1))

            # Input stream, strictly ordered on the sync ring in consumption
            # order.
            prev = None

            def chain(dma):
                nonlocal prev
                if prev is not None:
                    add_dep_helper(dma.ins, prev.ins, sync=False,
                                   reason="input stream in consumption order")
                prev = dma

            def hs0_dma(q0, q1):
                chain(nc.sync.dma_start(
                    hst0[:, q0:q1],
                    hs0_d.ap()[:, q0 * T * 3 * 128:q1 * T * 3 * 128]))

            def hs_dma(s, t):
                off = ((s - 1) * T + t) * NK * 128
                chain(nc.sync.dma_start(
                    hss[:, s - 1, t],
                    hs_d.ap()[:, off:off + NK * 128]))

            def chunk_part(v, k0, k1):
                chain(nc.sync.dma_start(
                    htall[:, k0:k1, v * NJ:(v + 1) * NJ],
                    ht_d.ap()[k0 * 128:k1 * 128, v * NJ:(v + 1) * NJ]
                    .rearrange("(k p) j -> p k j", p=128)))

            # Stream order: slot-0 stationaries and chunk-3 quarters lead
            # (they gate the opening), then the slot 1-3 stationaries in
            # consumption order, then chunks 2, 1, 0.
            hs0_dma(0, 1)
            chunk_part(3, 0, 3)
            hs0_dma(1, 4)
            chunk_part(3, 3, 6)
            chunk_part(3, 6, 9)
            chunk_part(3, 9, 12)
            for s in (1, 2, 3):
                for t in range(T):
                    hs_dma(s, t)
            for v in PHASES[1:]:
                for k0, k1 in KPARTS[v]:
                    chunk_part(v, k0, k1)

            def emit_partial(v, accs, ccv):
                q = (v + ccv) % 4
                off, fw = NJ * v + 128 * q, NJ - 128 * q
                stat = s0ap if v == 0 else \
                    (lambda t, k: sap(v, t, k))
                if v == 0:
                    # Final phase: split into 128-col chunks (separate PSUM
                    # accumulation groups) so stores drain progressively and
                    # the post-last-matmul chain is short.
                    nch = fw // 128
                    for t in range(T):
                        for c in range(nch):
                            for k in range(NK):
                                nc.tensor.matmul(
                                    accs[t][:, 128 * c:128 * (c + 1)],
                                    stat(t, k),
                                    htall[:, k, off + 128 * c:
                                          off + 128 * (c + 1)],
                                    start=(k == 0), stop=(k == NK - 1))
                else:
                    for t in range(T):
                        for k in range(NK):
                            nc.tensor.matmul(
                                accs[t][:, 0:fw], stat(t, k),
                                htall[:, k, off:off + fw],
                                start=(k == 0), stop=(k == NK - 1))

            def emit_static(s, v, acc, t):
                for k in range(NK):
                    nc.tensor.matmul(
                        acc[:], sap(s, t, k),
                        htall[:, k, v * NJ:(v + 1) * NJ],
                        start=(k == 0), stop=(k == NK - 1))

            def store(gi, acc):
                # Copies on scalar + DMA on the gpsimd software queue; the
                # gpsimd queue has a ~2.7us teardown drain, so it only
                # carries stores that complete well before the end.
                st = stp.tile([128, NJ], bf16, name="st", tag="st")
                nc.scalar.copy(st[:], acc[:])
                nc.gpsimd.dma_start(out_d.ap()[gi], st[:])

            def store_final(gi, acc, engsel):
                # 128-col chunk stores for the v=0 partial groups; the last
                # chunks ride vector copy + sync DMA (sync's teardown drain
                # is ~8ns) so the final chain after the last matmul is one
                # short copy + one small DMA.
                for c in range(4):
                    colr = slice(128 * c, 128 * (c + 1))
                    sth = stp.tile([128, 128], bf16, name="stf", tag="stf")
                    if engsel(c) == 0:
                        nc.scalar.copy(sth[:], acc[:, colr])
                        nc.scalar.dma_start(out_d.ap()[gi][:, colr], sth[:])
                    else:
                        nc.vector.tensor_copy(sth[:], acc[:, colr])
                        nc.sync.dma_start(out_d.ap()[gi][:, colr], sth[:])

            gi = 0
            for v in PHASES:
                first_s = 0
                if v > 0:
                    # Each phase's opening static pair can be paced by its
                    # chunk's stream: interleaving t0/t1 by k-part (two PSUM
                    # banks accumulating concurrently) matches consumption
                    # to arrival — free when the data is already resident.
                    accs0 = [psp.tile([128, NJ], f32, name="acc", tag="acc")
                             for _t in range(T)]
                    for k0, k1 in KPARTS[v]:
                        for t in range(T):
                            for k in range(k0, k1):
                                nc.tensor.matmul(
                                    accs0[t][:], s0ap(t, k),
                                    htall[:, k, v * NJ:(v + 1) * NJ],
                                    start=(k == 0), stop=(k == NK - 1))
                    for t in range(T):
                        store(gi, accs0[t])
                        gi += 1
                    first_s = 1
                for s in range(first_s, v):
                    for t in range(T):
                        acc = psp.tile([128, NJ], f32, name="acc", tag="acc")
                        emit_static(s, v, acc, t)
                        store(gi, acc)
                        gi += 1
                accs = [psp.tile([128, NJ], f32, name="acc", tag="acc")
                        for _t in range(T)]
                with tc.If(cc <= 1) as c1:
                    with tc.If(cc == 0) as c2:
                        emit_partial(v, accs, 0)
                    with c2.Else():
                        emit_partial(v, accs, 1)
                with c1.Else():
                    with tc.If(cc == 2) as c3:
                        emit_partial(v, accs, 2)
                    with c3.Else():
                        emit_partial(v, accs, 3)
                if v == 0:
                    # t0 chunks via scalar, t1 chunks alternate so the very
                    # last chunk drains on vector+sync.
                    store_final(gi, accs[0], lambda c: 0)
                    gi += 1
                    store_final(gi, accs[1], lambda c: 0 if c < 2 else 1)
                    gi += 1
                else:
                    for t in range(T):
                        store(gi, accs[t])
                        gi += 1
    nc.compile()
    _CACHE["nc"] = nc
    return nc


def make_in_maps(h, W):
    Wp = W[:, :D].astype(np.float32)
    # wpk[t, k, p] for the host-side premultiply
    wpk = Wp.reshape(T, NK, 128)
    hts = [np.ascontiguousarray(h[bi].T).astype(BF16) for bi in range(B)]
    in_maps = []
    for c in range(NCORES):
        bi, cc = c // CPB, c % CPB
        ht32 = hts[bi].astype(np.float32)

        def premul(s):
            # [t, k, p, r] = window_s[k, p, r] * wpk[t, k, p]
            I = row_of(s, cc)
            win = ht32[:, 128 * I:128 * I + 128].reshape(NK, 128, 128)
            return win[None] * wpk[:, :, :, None]

        # hs0: [t, k, p, r] -> [p, q=k//3, t, kk=k%3, r]
        hs0 = premul(0).reshape(T, 4, 3, 128, 128)
        hs0 = np.ascontiguousarray(hs0.transpose(3, 1, 0, 2, 4)).reshape(
            128, 4 * T * 3 * 128).astype(BF16)
        # hs: [s-1, t, k, p, r] -> [p, s-1, t, k, r]
        hs = np.stack([premul(s) for s in (1, 2, 3)])
        hs = np.ascontiguousarray(hs.transpose(3, 0, 1, 2, 4)).reshape(
            128, 3 * T * NK * 128).astype(BF16)
        in_maps.append({"ht": hts[bi], "hs0": hs0, "hs": hs})
    return in_maps


def kernel(hidden_states, W, b):
    from concourse.bass_utils import run_bass_kernel_spmd

    h = np.ascontiguousarray(hidden_states, dtype=np.float32)
    W = np.asarray(W, dtype=np.float32)
    bias = np.asarray(b, dtype=np.float32)
    nc = _get_nc()

    res = run_bass_kernel_spmd(nc, make_in_maps(h, W),
                               core_ids=list(range(NCORES)))
    full = np.empty((B, L, L, T), np.float32)
    for c in range(NCORES):
        bi, cc = c // CPB, c % CPB
        blocks = np.asarray(res.results[c]["out"]).astype(np.float32)
        for gi, (I, colstart, fw) in enumerate(groups_of(cc)):
            t = gi % T
            rows = slice(128 * I, 128 * I + 128)
            full[bi, rows, colstart:colstart + fw, t] = blocks[gi, :, 0:fw]
    # Mirror: keep computed j >= i, take j < i from the transpose.
    idx = np.arange(L)
    mask = (idx[None, :] >= idx[:, None])[None, :, :, None]
    out = np.where(mask, full, full.transpose(0, 2, 1, 3))
    if np.any(bias != 0):
        out += bias
    return out


# revision 14
# speedup vs baseline: 1.3351x; 1.0671x over previous
"""Trainium2 Bass kernel for nn_ContactPredictionHead.

Reference computation (B=2, L=2048, D=1536, T=2):
    Wp, Wd = W[:, :D], W[:, D:]
    prod[b,i,j,t] = sum_d h[b,i,d] * Wp[t,d] * h[b,j,d]
    diff[b,i,j,t] = (h@Wd.T)[b,i,t] - (h@Wd.T)[b,j,t]
    out = symmetrize(prod + diff + bias)

Key identity: prod is symmetric in (i,j) and diff is antisymmetric, so the
symmetrization leaves   out[b,i,j,t] = prod[b,i,j,t] + bias[t]   exactly —
a weighted Gram matrix.  Only the upper triangle is computed on device; the
host mirrors the strict lower triangle.

Sharding: 4 cores per batch item.  The 16 row-blocks (128 rows each) of a
batch's L x L Gram matrix are dealt by a Latin square: core cc's stationary
slot s holds row-block I = 4s + ((s+cc)%4).  Slot s covers its arc
[128I, 2048) as one cc-dependent "partial" group [128I, 512(s+1)) plus
(3-s) full 512-col "static" groups — so every core computes exactly
4352 moving columns per t (the balanced ideal).  Static groups are
identical on all cores; the partial groups' offsets/sizes live in a 4-way
partition-id branch on the Tensor engine only (all other engines run
straight-line code: PSUM accs are padded to 512 and the host slices each
group's valid columns).

Phases run in REVERSE chunk order (3,2,1,0): per-phase matmul work then
always exceeds the per-chunk stream time, so the PE can never starve on
the input stream after phase 3's opening.

ALL weighted stationaries (slot window x Wp[t]) are PRE-MULTIPLIED ON THE
HOST and shipped p-major contiguous, so no on-device prep exists at all:
every matmul gates only on plain DMA arrivals, and the Vector/Scalar
engines carry nothing but output copies.  Slot 0 (which feeds every
phase's opening statics) is k-quartered so the first matmul starts on
~0.5MB of stream.

The PE clock is HAM-throttled to 1.2 GHz until ~3.4us of SUSTAINED
activity; the warmup block issues 512-col dummy matmuls (one chained
accumulation group, so they issue back-to-back) on a locally-memset tile
to bridge from kernel start to the first real matmul.

All tensors stream as bfloat16 (PSUM accumulates fp32), halving DMA
volume vs fp32 at the same PE rate.  The final (v=0) partial's stores are
split into 128-col chunks so the post-last-matmul store drain is short.
"""
import sys

sys.path.insert(0, "/opt/trn_rl_repo")

import numpy as np
import ml_dtypes

BF16 = ml_dtypes.bfloat16

B, L, D, T = 2, 2048, 1536, 2
NCORES = 8
CPB = NCORES // B     # cores per batch item = 4
NK = D // 128         # contraction k-tiles = 12
NJ = 512              # j columns per full matmul (one PSUM bank of fp32)
NNB = L // NJ         # j chunks = 4
NS = 4                # stationary row slots per core (128 rows each)

PHASES = [3, 2, 1, 0]           # chunk/phase order (heaviest work first)
KPARTS = {3: [(0, 3), (3, 6), (6, 9), (9, 12)],
          2: [(0, 6), (6, 12)],
          1: [(0, 6), (6, 12)],
          0: [(0, 6), (6, 12)]}
WARMUP_MMS = 13                 # 512-col dummies, one accumulation group


def row_of(s, cc):
    """Global 128-row block held by slot s on a core with variant cc."""
    return 4 * s + (s + cc) % 4


def groups_of(cc):
    """Schedule (shared shape, variant-dependent geometry): list of
    (I, colstart, F) in emission order; 20 groups = [statics (s<v) t0, t1,
    then partial t0, t1] per chunk phase v, phases in PHASES order."""
    gs = []
    for v in PHASES:
        q = (v + cc) % 4
        for s in range(v):
            for _t in range(T):
                gs.append((row_of(s, cc), NJ * v, NJ))
        for _t in range(T):
            gs.append((row_of(v, cc), NJ * v + 128 * q, NJ - 128 * q))
    return gs


NG = len(groups_of(0))    # 20

_CACHE = {}


def _get_nc():
    if "nc" in _CACHE:
        return _CACHE["nc"]
    import concourse.tile as tile
    from concourse.tile_rust import add_dep_helper
    from concourse import bacc, mybir

    f32, bf16 = mybir.dt.float32, mybir.dt.bfloat16
    nc = bacc.Bacc("TRN2", target_bir_lowering=False, debug=False,
                   num_devices=NCORES, enable_partition_id=True,
                   enable_asserts=False)
    ht_d = nc.dram_tensor("ht", [D, L], bf16, kind="ExternalInput")
    # hs0[p, (q, t, kk, r)]: slot-0 stationaries, k-quartered (q=k//3):
    #   ht[128(3q+kk)+p, 128*row_of(0,cc)+r] * Wp[t, 128(3q+kk)+p]
    hs0_d = nc.dram_tensor("hs0", [128, 4 * T * 3 * 128], bf16,
                           kind="ExternalInput")
    # hs[p, (s-1, t, k, r)]: slot 1..3 stationaries (host premultiplied)
    hs_d = nc.dram_tensor("hs", [128, 3 * T * NK * 128], bf16,
                          kind="ExternalInput")
    out_d = nc.dram_tensor("out", [NG, 128, NJ], bf16, kind="ExternalOutput")

    with tile.TileContext(nc) as tc:
        with tc.tile_pool(name="big", bufs=1) as big, \
             tc.tile_pool(name="st", bufs=4) as stp, \
             tc.tile_pool(name="ps", bufs=6, space="PSUM") as psp, \
             tc.tile_pool(name="psw", bufs=1, space="PSUM") as psw:
            # hst0[p, q, t, kk, r]: slot-0 stationaries (k-quartered)
            hst0 = big.tile([128, 4, T, 3, 128], bf16, name="hst0")
            # hss[p, s-1, t, k, r]: slot 1..3 stationaries
            hss = big.tile([128, 3, T, NK, 128], bf16, name="hss")
            # htall[p, k, j] = ht[128k+p, j]  (canonical, un-rolled)
            htall = big.tile([128, NK, L], bf16, name="htall")

            def s0ap(t, k):
                return hst0[:, k // 3, t, k % 3]

            def sap(s, t, k):
                return hss[:, s - 1, t, k]

            # Partition-id register load costs ~1.4us of queue time — issue
            # it first so it overlaps the framework preamble and DMA issues
            # instead of delaying the branch evaluations later.
            pid = nc.tensor.partition_id()
            cc = pid % 4

            # Warm the PE clock (HAM un-throttles after ~3.4 us of SUSTAINED
            # activity) with 512-col throwaway matmuls on a
            # locally-initialized scratch tile — no DMA dependency, so
            # warmup bridges from the preamble to the first real matmul.
            # One accumulation group: back-to-back issue, no PSUM WAW stalls.
            wdum = big.tile([128, NJ], bf16, name="wdum")
            nc.gpsimd.memset(wdum[:], 0.0)
            wacc = psw.tile([128, NJ], f32, name="wacc")
            for i in range(WARMUP_MMS):
                nc.tensor.matmul(wacc[:], wdum[:, 0:128], wdum[:],
                                 start=(i == 0), stop=(i == WARMUP_MMS - 1))

            # Input stream, strictly ordered on the sync ring in consumption
            # order.
            prev = None

            def chain(dma):
                nonlocal prev
                if prev is not None:
                    add_dep_helper(dma.ins, prev.ins, sync=False,
                                   reason="input stream in consumption order")
                prev = dma

            def hs0_dma(q0, q1):
                chain(nc.sync.dma_start(
                    hst0[:, q0:q1],
                    hs0_d.ap()[:, q0 * T * 3 * 128:q1 * T * 3 * 128]))

            def hs_dma(s, t):
                off = ((s - 1) * T + t) * NK * 128
                chain(nc.sync.dma_start(
                    hss[:, s - 1, t],
                    hs_d.ap()[:, off:off + NK * 128]))

            def chunk_part(v, k0, k1):
                chain(nc.sync.dma_start(
                    htall[:, k0:k1, v * NJ:(v + 1) * NJ],
                    ht_d.ap()[k0 * 128:k1 * 128, v * NJ:(v + 1) * NJ]
                    .rearrange("(k p) j -> p k j", p=128)))

            # Stream order: the phase-3 opening is stream-bound, so slot
            # stationaries are interleaved with the chunk-3 k-quarters in
            # rotation-consumption order — each arriving k-quarter is
            # chewed by every already-resident (slot, t) group, keeping
            # the PE fed at the stream rate.  Then chunks 2, 1, 0.
            hs0_dma(0, 1)
            chunk_part(3, 0, 3)
            hs0_dma(1, 4)
            hs_dma(1, 0)
            chunk_part(3, 3, 6)
            hs_dma(1, 1)
            chunk_part(3, 6, 9)
            hs_dma(2, 0)
            chunk_part(3, 9, 12)
            hs_dma(2, 1)
            hs_dma(3, 0)
            hs_dma(3, 1)
            for v in PHASES[1:]:
                for k0, k1 in KPARTS[v]:
                    chunk_part(v, k0, k1)

            def emit_partial(v, accs, ccv):
                q = (v + ccv) % 4
                off, fw = NJ * v + 128 * q, NJ - 128 * q
                stat = s0ap if v == 0 else \
                    (lambda t, k: sap(v, t, k))
                for t in range(T):
                    for k in range(NK):
                        nc.tensor.matmul(
                            accs[t][:, 0:fw], stat(t, k),
                            htall[:, k, off:off + fw],
                            start=(k == 0), stop=(k == NK - 1))

            def emit_static(s, v, acc, t):
                for k in range(NK):
                    nc.tensor.matmul(
                        acc[:], sap(s, t, k),
                        htall[:, k, v * NJ:(v + 1) * NJ],
                        start=(k == 0), stop=(k == NK - 1))

            def store(gi, acc):
                # Copies on scalar: vector stays free until the end.  The
                # last two groups (the v=0 partials) route around gpsimd,
                # whose teardown drain is ~2.7us: t0's chain hides inside
                # t1's matmuls; t1's store splits into two halves on
                # parallel vector+sync / scalar+scalar chains (sync's
                # drain is ~8ns).
                if gi == NG - 2:
                    st = stp.tile([128, NJ], bf16, name="st", tag="st")
                    nc.vector.tensor_copy(st[:], acc[:])
                    nc.sync.dma_start(out_d.ap()[gi], st[:])
                elif gi == NG - 1:
                    # The tile framework serializes the second-emitted
                    # reader of a PSUM tile behind the first, so the longer
                    # chain (vector CAST -> sync DMA) goes first.
                    for hh in (1, 0):
                        sth = stp.tile([128, NJ // 2], bf16, name="sth",
                                       tag="sth")
                        half = slice(hh * (NJ // 2), (hh + 1) * (NJ // 2))
                        if hh == 0:
                            nc.scalar.copy(sth[:], acc[:, half])
                            nc.scalar.dma_start(out_d.ap()[gi][:, half],
                                                sth[:])
                        else:
                            nc.vector.tensor_copy(sth[:], acc[:, half])
                            nc.sync.dma_start(out_d.ap()[gi][:, half],
                                              sth[:])
                else:
                    st = stp.tile([128, NJ], bf16, name="st", tag="st")
                    nc.scalar.copy(st[:], acc[:])
                    nc.gpsimd.dma_start(out_d.ap()[gi], st[:])

            def mmrange(acc, stat_fn, t, k0, k1, cols):
                for k in range(k0, k1):
                    nc.tensor.matmul(
                        acc[:], stat_fn(t, k), htall[:, k, cols],
                        start=(k == 0), stop=(k == NK - 1))

            gi = 0
            for v in PHASES:
                first_s = 0
                if v == 3:
                    # Phase-3 rotation: the opening is stream-bound, so the
                    # s0 pair and the s1 pair chew each chunk-3 k-quarter
                    # as it arrives, in stream order (see the DMA chain).
                    cols = slice(v * NJ, (v + 1) * NJ)
                    acc_s0 = [psp.tile([128, NJ], f32, name="acc",
                                       tag="acc") for _t in range(T)]
                    acc_s1 = [psp.tile([128, NJ], f32, name="acc",
                                       tag="acc") for _t in range(T)]
                    s1 = lambda t, k: sap(1, t, k)
                    for t in range(T):
                        mmrange(acc_s0[t], s0ap, t, 0, 3, cols)    # q0
                    for t in range(T):
                        mmrange(acc_s0[t], s0ap, t, 3, 6, cols)    # q1
                    mmrange(acc_s1[0], s1, 0, 0, 6, cols)
                    for t in range(T):
                        mmrange(acc_s0[t], s0ap, t, 6, 9, cols)    # q2
                    mmrange(acc_s1[1], s1, 1, 0, 9, cols)
                    mmrange(acc_s1[0], s1, 0, 6, 9, cols)
                    for t in range(T):
                        mmrange(acc_s0[t], s0ap, t, 9, 12, cols)   # q3
                    mmrange(acc_s1[0], s1, 0, 9, 12, cols)
                    mmrange(acc_s1[1], s1, 1, 9, 12, cols)
                    for t in range(T):
                        store(gi, acc_s0[t])
                        gi += 1
                    for t in range(T):
                        store(gi, acc_s1[t])
                        gi += 1
                    first_s = 2
                elif v > 0:
                    # Later phases' opening static pair: interleave t0/t1
                    # by k-half for robustness under stream contention.
                    accs0 = [psp.tile([128, NJ], f32, name="acc", tag="acc")
                             for _t in range(T)]
                    for k0, k1 in KPARTS[v]:
                        for t in range(T):
                            for k in range(k0, k1):
                                nc.tensor.matmul(
                                    accs0[t][:], s0ap(t, k),
                                    htall[:, k, v * NJ:(v + 1) * NJ],
                                    start=(k == 0), stop=(k == NK - 1))
                    for t in range(T):
                        store(gi, accs0[t])
                        gi += 1
                    first_s = 1
                for s in range(first_s, v):
                    for t in range(T):
                        acc = psp.tile([128, NJ], f32, name="acc", tag="acc")
                        emit_static(s, v, acc, t)
                        store(gi, acc)
                        gi += 1
                accs = [psp.tile([128, NJ], f32, name="acc", tag="acc")
                        for _t in range(T)]
                with tc.If(cc <= 1) as c1:
                    with tc.If(cc == 0) as c2:
                        emit_partial(v, accs, 0)
                    with c2.Else():
                        emit_partial(v, accs, 1)
                with c1.Else():
                    with tc.If(cc == 2) as c3:
                        emit_partial(v, accs, 2)
                    with c3.Else():
                        emit_partial(v, accs, 3)
                for t in range(T):
                    store(gi, accs[t])
                    gi += 1
    nc.compile()
    _CACHE["nc"] = nc
    return nc


def make_in_maps(h, W):
    Wp = W[:, :D].astype(np.float32)
    # wpk[t, k, p] for the host-side premultiply
    wpk = Wp.reshape(T, NK, 128)
    hts = [np.ascontiguousarray(h[bi].T).astype(BF16) for bi in range(B)]
    in_maps = []
    for c in range(NCORES):
        bi, cc = c // CPB, c % CPB
        ht32 = hts[bi].astype(np.float32)

        def premul(s):
            # [t, k, p, r] = window_s[k, p, r] * wpk[t, k, p]
            I = row_of(s, cc)
            win = ht32[:, 128 * I:128 * I + 128].reshape(NK, 128, 128)
            return win[None] * wpk[:, :, :, None]

        # hs0: [t, k, p, r] -> [p, q=k//3, t, kk=k%3, r]
        hs0 = premul(0).reshape(T, 4, 3, 128, 128)
        hs0 = np.ascontiguousarray(hs0.transpose(3, 1, 0, 2, 4)).reshape(
            128, 4 * T * 3 * 128).astype(BF16)
        # hs: [s-1, t, k, p, r] -> [p, s-1, t, k, r]
        hs = np.stack([premul(s) for s in (1, 2, 3)])
        hs = np.ascontiguousarray(hs.transpose(3, 0, 1, 2, 4)).reshape(
            128, 3 * T * NK * 128).astype(BF16)
        in_maps.append({"ht": hts[bi], "hs0": hs0, "hs": hs})
    return in_maps


def kernel(hidden_states, W, b):
    from concourse.bass_utils import run_bass_kernel_spmd

    h = np.ascontiguousarray(hidden_states, dtype=np.float32)
    W = np.asarray(W, dtype=np.float32)
    bias = np.asarray(b, dtype=np.float32)
    nc = _get_nc()

    res = run_bass_kernel_spmd(nc, make_in_maps(h, W),
                               core_ids=list(range(NCORES)))
    full = np.empty((B, L, L, T), np.float32)
    for c in range(NCORES):
        bi, cc = c // CPB, c % CPB
        blocks = np.asarray(res.results[c]["out"]).astype(np.float32)
        for gi, (I, colstart, fw) in enumerate(groups_of(cc)):
            t = gi % T
            rows = slice(128 * I, 128 * I + 128)
            full[bi, rows, colstart:colstart + fw, t] = blocks[gi, :, 0:fw]
    # Mirror: keep computed j >= i, take j < i from the transpose.
    idx = np.arange(L)
    mask = (idx[None, :] >= idx[:, None])[None, :, :, None]
    out = np.where(mask, full, full.transpose(0, 2, 1, 3))
    if np.any(bias != 0):
        out += bias
    return out
